# revision 18
# baseline (speedup 1.0000x reference)
"""Trainium2 Bass kernel for nn_ByteToLatentAttention.

Sharding: 8 cores = 2 (batch) x 4 (head-groups of 4 heads).  Each core
computes a partial output  attn_part @ wout_rows + merged_raw_rows @ wbyp_rows
for its batch; the host sums the 4 partials per batch and adds wout_b.

v3: the softmax exp stream on ScalarE (~128us/core at 1 elem/cyc/lane) is the
hard floor; the kernel saturates it from ~16us on.  Key tricks:
 - x^T columns (and the K rope tables) are host-permuted to sub-major order
   (s' = (s%4)*LQ + s//4).  Softmax/attention are invariant to a consistent
   permutation of the k axis, and the Q projection's rhs becomes contiguous
   (strided rhs ran matmuls 3x slow).
 - minimal prefix: chunked x DMA -> RMSnorm (ACT Rsqrt) -> K/V head-pair 0
   chunks 0-1 -> Q(hp0,qc0); then 4 attention blocks with every remaining
   projection / V chunk / norm tail / first out-proj interleaved between
   attention matmuls as small closures with explicit emit iterations.
 - during attention ScalarE runs exp only; support ops go to DVE.

Self-contained: hardcodes all shapes; uses only numpy + concourse.
"""

from contextlib import ExitStack

import numpy as np

import concourse.bass as bass
import concourse.tile as tile
from concourse import bacc
from concourse import mybir
from concourse.bass_utils import run_bass_kernel_spmd
from concourse.masks import make_identity

# ---- problem constants ----
B, S, D = 2, 4096, 512
BPL, H, DQK = 4, 16, 64
DLAT = 1024
LQ = S // BPL  # 1024
EPS = 1.1920929e-07
ROPE_BASE = 10000.0
NCORES = 8
NH = (H // 4) * DQK  # 256 features per core (4 heads)
P = 128

F32 = mybir.dt.float32
BF16 = mybir.dt.bfloat16
MM_F32 = mybir.dt.float32r  # full-rate PE path for 4-byte data

AF = mybir.ActivationFunctionType
ALU = mybir.AluOpType
AX = mybir.AxisListType


def _kernel_body(ctx: ExitStack, tc, io):
    nc = tc.nc

    const = ctx.enter_context(tc.tile_pool(name="const", bufs=1))
    work = ctx.enter_context(tc.tile_pool(name="work", bufs=2))
    evp = ctx.enter_context(tc.tile_pool(name="evp", bufs=6))
    psP = ctx.enter_context(tc.tile_pool(name="psP", bufs=2, space="PSUM"))
    psS = ctx.enter_context(tc.tile_pool(name="psS", bufs=2, space="PSUM"))
    psAcc = ctx.enter_context(tc.tile_pool(name="psAcc", bufs=1, space="PSUM"))
    psDen = ctx.enter_context(tc.tile_pool(name="psDen", bufs=1, space="PSUM"))

    # ---------------- constants / persistent tiles ----------------
    ones128 = const.tile([P, P], BF16)
    nc.vector.memset(ones128, 1.0)
    ones64 = const.tile([P, 64], BF16)
    nc.vector.memset(ones64, 1.0)
    eps_sb = const.tile([P, 1], F32)
    nc.vector.memset(eps_sb, EPS)

    rot_sb = const.tile([P, P], BF16)
    bias_sb = const.tile([P, 6], F32)  # cols 0-1 bk, 2-3 bq, 4-5 bv
    cs_k = const.tile([P, 2, S], BF16)
    cs_q = const.tile([P, 2, LQ], BF16)

    xT = const.tile([P, 4, S], BF16)      # normalized in place after norm
    bypT = const.tile([P, 4, LQ], MM_F32)
    KTr = const.tile([P, 2, S], BF16)
    QTr = const.tile([P, 2, LQ], BF16)
    Vn = const.tile([P, 32, NH], BF16)
    acT = const.tile([P, 2, LQ], BF16)
    rinvh = const.tile([P, 8, 512], BF16)  # per-chunk inverse rms (bf16)

    wq_sb = const.tile([P, 16, NH], BF16)
    wk_sb = const.tile([P, 4, NH], BF16)
    wv_sb = const.tile([P, 4, NH], BF16)
    wo_sb = const.tile([P, 2, DLAT], BF16)
    wb_sb = const.tile([P, 4, DLAT], MM_F32)

    bk_sb = bias_sb[:, 0:2]
    bq_sb = bias_sb[:, 2:4]
    bv_sb = bias_sb[:, 4:6]

    # ---------------- DMA issue (priority order; queues run parallel) ----
    # even x chunks first: Q(hp0,qc0) reads chunks 0,2,4,6 (sub-major layout)
    dma = nc.sync.dma_start

    def dma_x(c):
        dma(out=xT[:, :, c * 512 : (c + 1) * 512],
            in_=io["x_b"][:, :, c * 512 : (c + 1) * 512])

    dma_x(0)
    dma(out=rot_sb, in_=io["rotm"])
    dma(out=bias_sb, in_=io["biases"])
    dma_x(2)
    dma(out=wk_sb, in_=io["wk"])
    dma(out=cs_k[:, :, 0:2048], in_=io["csk"][:, :, 0:2048])
    dma_x(4)
    dma(out=wq_sb, in_=io["wq"])
    dma(out=cs_q, in_=io["csq"])
    dma_x(6)
    dma(out=wv_sb, in_=io["wv"])
    dma_x(1)
    dma_x(3)
    dma(out=cs_k[:, :, 2048:4096], in_=io["csk"][:, :, 2048:4096])
    dma_x(5)
    dma_x(7)
    dma(out=wo_sb, in_=io["wo"])
    dma(out=wb_sb, in_=io["wb"])
    dma(out=bypT, in_=io["x_byp"])

    cosk_sb = cs_k[:, 0, :]
    sink_sb = cs_k[:, 1, :]
    cosq_sb = cs_q[:, 0, :]
    sinq_sb = cs_q[:, 1, :]

    # ---------------- PE warm-up (HAM un-throttle) ----------------
    warm_ps = psS.tile([P, 1024], F32, tag="sc")

    def warm(n):
        for _ in range(n):
            nc.tensor.matmul(warm_ps[:, 0:128], lhsT=ones128, rhs=ones128,
                             start=True, stop=True)

    warm(20)

    # ---------------- emit helpers ----------------
    def norm_stats(c):
        """squares (2 ACT + 2 DVE) + sumsq MMs (PE) + Rsqrt (ACT, bf16 out)."""
        ssl = slice(c * 512, (c + 1) * 512)
        pss = psP.tile([P, 512], F32, tag="mm")
        for dc in range(4):
            sq = work.tile([P, 512], BF16, tag="sq")
            if dc < 2:
                nc.scalar.square(out=sq, in_=xT[:, dc, ssl])
            else:
                nc.vector.tensor_mul(out=sq, in0=xT[:, dc, ssl], in1=xT[:, dc, ssl])
            nc.tensor.matmul(pss, lhsT=ones128, rhs=sq, start=(dc == 0), stop=(dc == 3))
        rmsb = work.tile([P, 512], F32, tag="rmsb")
        nc.scalar.activation(out=rmsb, in_=pss, func=AF.Sqrt,
                             bias=eps_sb, scale=1.0 / D)
        rinvf = work.tile([P, 512], F32, tag="rinvf")
        nc.vector.reciprocal_approx_fast(out=rinvf, in_=rmsb)
        nc.vector.tensor_copy(out=rinvh[:, c, :], in_=rinvf)

    def norm_mul(c):
        ssl = slice(c * 512, (c + 1) * 512)
        for dc in range(4):
            nc.vector.tensor_mul(out=xT[:, dc, ssl], in0=xT[:, dc, ssl],
                                 in1=rinvh[:, c, :])

    def k_proj(sf, mk, holder):
        """part A: 4 accumulating proj MMs (psum handle -> holder)."""
        ssl = slice(sf * 512, (sf + 1) * 512)
        pk = psP.tile([P, 512], F32, tag="mm")
        holder[0] = pk
        for dc in range(4):
            nc.tensor.matmul(
                pk, lhsT=wk_sb[:, dc, mk * P : (mk + 1) * P], rhs=xT[:, dc, ssl],
                start=(dc == 0), stop=(dc == 3),
            )

    def k_rope_a(sf, mk, holder, scalar_eng):
        """part B: bias-add (psum->sbuf) + rot MM; psum handle kept."""
        pk = holder[0]
        kb = work.tile([P, 512], BF16, tag="qb")
        if scalar_eng:
            nc.scalar.add(out=kb, in_=pk, add=bk_sb[:, mk : mk + 1])
        else:
            nc.vector.tensor_scalar_add(out=kb, in0=pk, scalar1=bk_sb[:, mk : mk + 1])
        pr = psP.tile([P, 512], F32, tag="mm")
        nc.tensor.matmul(pr, lhsT=rot_sb, rhs=kb, start=True, stop=True)
        holder[0] = pr
        holder.append(kb)

    def k_rope_b(sf, mk, holder, scalar_eng):
        """part C: rotated copy + cos/sin combine."""
        ssl = slice(sf * 512, (sf + 1) * 512)
        pr, kb = holder[0], holder[1]
        prh = work.tile([P, 512], BF16, tag="prh")
        if scalar_eng:
            nc.scalar.copy(out=prh, in_=pr)
        else:
            nc.vector.tensor_copy(out=prh, in_=pr)
        t1 = work.tile([P, 512], BF16, tag="t1")
        nc.vector.tensor_mul(out=t1, in0=kb, in1=cosk_sb[:, ssl])
        t2 = work.tile([P, 512], BF16, tag="t2")
        nc.vector.tensor_mul(out=t2, in0=prh, in1=sink_sb[:, ssl])
        nc.vector.tensor_add(out=KTr[:, mk, ssl], in0=t1, in1=t2)

    def k_chunk(sf, mk, scalar_eng):
        h = [None]
        k_proj(sf, mk, h)
        k_rope_a(sf, mk, h, scalar_eng)
        k_rope_b(sf, mk, h, scalar_eng)

    def q_proj_half(qf, mq, half, holder):
        """8 accumulating Q MMs (half 0: subs 0-1, half 1: subs 2-3)."""
        if half == 0:
            pq = psP.tile([P, 512], F32, tag="mm")
            holder[0] = pq
        else:
            pq = holder[0]
        for sub in range(2 * half, 2 * half + 2):
            for dc in range(4):
                kc = sub * 4 + dc
                rhs = xT[:, dc, sub * LQ + qf * 512 : sub * LQ + (qf + 1) * 512]
                nc.tensor.matmul(
                    pq, lhsT=wq_sb[:, kc, mq * P : (mq + 1) * P], rhs=rhs,
                    start=(kc == 0), stop=(kc == 15),
                )

    def q_rope(qf, mq, holder, scalar_eng):
        qsl = slice(qf * 512, (qf + 1) * 512)
        pq = holder[0]
        qb = work.tile([P, 512], BF16, tag="qb")
        if scalar_eng:
            nc.scalar.add(out=qb, in_=pq, add=bq_sb[:, mq : mq + 1])
        else:
            nc.vector.tensor_scalar_add(out=qb, in0=pq, scalar1=bq_sb[:, mq : mq + 1])
        pr = psP.tile([P, 512], F32, tag="mm")
        nc.tensor.matmul(pr, lhsT=rot_sb, rhs=qb, start=True, stop=True)
        prh = work.tile([P, 512], BF16, tag="prh")
        if scalar_eng:
            nc.scalar.copy(out=prh, in_=pr)
        else:
            nc.vector.tensor_copy(out=prh, in_=pr)
        t1 = work.tile([P, 512], BF16, tag="t1")
        nc.vector.tensor_mul(out=t1, in0=qb, in1=cosq_sb[:, qsl])
        t2 = work.tile([P, 512], BF16, tag="t2")
        nc.vector.tensor_mul(out=t2, in0=prh, in1=sinq_sb[:, qsl])
        nc.vector.tensor_add(out=QTr[:, mq, qsl], in0=t1, in1=t2)

    def q_group(qf, mq, scalar_eng):
        h = [None]
        q_proj_half(qf, mq, 0, h)
        q_proj_half(qf, mq, 1, h)
        q_rope(qf, mq, h, scalar_eng)

    def v_chunk(sc, hp, scalar_eng):
        """V projection for s'-chunk sc (128 rows), head-pair hp columns."""
        csl = slice(hp * P, (hp + 1) * P)
        pv = psP.tile([P, P], F32, tag="mm")
        for dc in range(4):
            nc.tensor.matmul(
                pv, lhsT=xT[:, dc, sc * P : (sc + 1) * P], rhs=wv_sb[:, dc, csl],
                start=(dc == 0), stop=(dc == 3),
            )
        if scalar_eng:
            nc.scalar.copy(out=Vn[:, sc, csl], in_=pv)
        else:
            nc.vector.tensor_copy(out=Vn[:, sc, csl], in_=pv)

    def outproj_a(q8, oc, ph):
        qsl8 = slice(q8 * P, (q8 + 1) * P)
        osl = slice(oc * 512, (oc + 1) * 512)
        po = psP.tile([P, 512], F32, tag="mm")
        ph[0] = po
        nc.tensor.matmul(po, lhsT=acT[:, 0, qsl8], rhs=wo_sb[:, 0, osl],
                         start=True, stop=False)
        nc.tensor.matmul(po, lhsT=acT[:, 1, qsl8], rhs=wo_sb[:, 1, osl],
                         start=False, stop=False)
        nc.tensor.matmul(po, lhsT=bypT[:, 0, qsl8], rhs=wb_sb[:, 0, osl],
                         start=False, stop=False)

    def outproj_b(q8, oc, ph, osb, vec_eng):
        qsl8 = slice(q8 * P, (q8 + 1) * P)
        osl = slice(oc * 512, (oc + 1) * 512)
        po = ph[0]
        for dc in range(1, 4):
            nc.tensor.matmul(po, lhsT=bypT[:, dc, qsl8], rhs=wb_sb[:, dc, osl],
                             start=False, stop=(dc == 3))
        if vec_eng:
            nc.vector.tensor_copy(out=osb[:, osl], in_=po)
        else:
            nc.scalar.copy(out=osb[:, osl], in_=po)
        if oc == 1:
            nc.sync.dma_start(out=io["out_partial"][qsl8, :], in_=osb)

    # ---------------- attention block ----------------
    def attention_block(hp, qc, closures, pinned_v=None, esum_mode=False):
        """32 sc iterations.  closures: list of (emit_iter, fn), ordered;
        emitted when iter >= emit_iter (between exp and attnV matmuls).
        pinned_v: head-pair whose v_chunk(sc+2) is emitted at iter sc.
        esum_mode: denominator via DVE pair-sum + den MM every 2 iters
        (DVE-heavy); else den MM pair every iter directly on eab (PE-heavy)."""
        qsl = slice(qc * 512, (qc + 1) * 512)
        pac = psAcc.tile([P, 512], F32, tag="pac")
        pden = psDen.tile([P, 512], F32, tag="pden")
        nsc = S // P
        ci = 0
        eprev = None
        for sc in range(nsc):
            ksl = slice(sc * P, (sc + 1) * P)
            psab = psS.tile([P, 1024], F32, tag="sc")
            psa = psab[:, 0:512]
            psb = psab[:, 512:1024]
            nc.tensor.matmul(psa, lhsT=KTr[0:64, hp, ksl], rhs=QTr[0:64, hp, qsl],
                             start=True, stop=True, skip_group_check=True)
            nc.tensor.matmul(psb, lhsT=KTr[64:128, hp, ksl], rhs=QTr[64:128, hp, qsl],
                             start=True, stop=True, skip_group_check=True)
            eab = evp.tile([P, 1024], BF16, tag="ea")
            nc.scalar.activation(out=eab, in_=psab, func=AF.Exp, scale=0.125)
            if pinned_v is not None and sc + 2 < nsc:
                v_chunk(sc + 2, pinned_v, scalar_eng=False)
            while ci < len(closures) and closures[ci][0] <= sc:
                closures[ci][1]()
                ci += 1
            ea = eab[:, 0:512]
            eb = eab[:, 512:1024]
            st, sp = (sc == 0), (sc == nsc - 1)
            cA = slice((2 * hp) * 64, (2 * hp) * 64 + 64)
            cB = slice((2 * hp + 1) * 64, (2 * hp + 1) * 64 + 64)
            nc.tensor.matmul(pac[0:64, :], lhsT=Vn[:, sc, cA], rhs=ea,
                             start=st, stop=sp, tile_position=(0, 0),
                             skip_group_check=True)
            nc.tensor.matmul(pac[64:128, :], lhsT=Vn[:, sc, cB], rhs=eb,
                             start=st, stop=sp, tile_position=(0, 64),
                             skip_group_check=True)
            if not esum_mode:
                nc.tensor.matmul(pden[0:64, :], lhsT=ones64, rhs=ea,
                                 start=st, stop=sp, tile_position=(0, 0),
                                 skip_group_check=True)
                nc.tensor.matmul(pden[64:128, :], lhsT=ones64, rhs=eb,
                                 start=st, stop=sp, tile_position=(0, 64),
                                 skip_group_check=True)
            elif sc % 2 == 0:
                eprev = eab
            else:
                esum = work.tile([P, 1024], BF16, tag="esum")
                nc.vector.tensor_add(out=esum, in0=eprev, in1=eab)
                dst, dsp = (sc == 1), (sc == nsc - 1)
                nc.tensor.matmul(pden[0:64, :], lhsT=ones64, rhs=esum[:, 0:512],
                                 start=dst, stop=dsp, tile_position=(0, 0),
                                 skip_group_check=True)
                nc.tensor.matmul(pden[64:128, :], lhsT=ones64, rhs=esum[:, 512:1024],
                                 start=dst, stop=dsp, tile_position=(0, 64),
                                 skip_group_check=True)
        while ci < len(closures):
            closures[ci][1]()
            ci += 1
        bc = work.tile([P, 512], F32, tag="bc")
        nc.vector.reciprocal_approx_fast(out=bc, in_=pden)
        tn = work.tile([P, 512], F32, tag="tn")
        nc.vector.tensor_mul(out=tn, in0=pac, in1=bc)
        nc.vector.tensor_scalar_add(out=acT[:, hp, qsl], in0=tn,
                                    scalar1=bv_sb[:, hp : hp + 1])

    # ================= prefix =================
    # Even chunks first (q(0,0) reads 0,2,4,6 in the sub-major layout);
    # odd-chunk stats (ACT Sqrt) must still precede the first exp; odd muls
    # become block-1 closures.  Warm filler MMs bridge DMA waits.
    norm_stats(0)
    norm_mul(0)
    norm_stats(2)
    norm_mul(2)
    warm(6)
    k_chunk(0, 0, scalar_eng=True)
    v_chunk(0, 0, scalar_eng=True)
    v_chunk(1, 0, scalar_eng=True)
    norm_stats(4)
    norm_mul(4)
    warm(6)
    norm_stats(6)
    norm_mul(6)
    q_group(0, 0, scalar_eng=True)
    norm_stats(1)
    norm_stats(3)
    warm(6)
    norm_stats(5)
    norm_stats(7)

    # ================= attention block 1: (hp0, qc0) =================
    # k0 chunks 1-7 JIT (chunk sf feeds scores iters 4sf+), odd norm-muls,
    # Q(hp0,qc1) split in two.
    hs0 = {sf: [None] for sf in range(1, 8)}
    hq = [None]
    cl = [
        (0, lambda: norm_mul(1)),
        (0, lambda: k_proj(1, 0, hs0[1])),
        (0, lambda: k_rope_a(1, 0, hs0[1], False)),
        (1, lambda: k_rope_b(1, 0, hs0[1], False)),
        (2, lambda: norm_mul(3)),
        (3, lambda: k_proj(2, 0, hs0[2])),
        (4, lambda: k_rope_a(2, 0, hs0[2], False)),
        (5, lambda: k_rope_b(2, 0, hs0[2], False)),
        (6, lambda: k_proj(3, 0, hs0[3])),
        (7, lambda: k_rope_a(3, 0, hs0[3], False)),
        (8, lambda: k_rope_b(3, 0, hs0[3], False)),
        (9, lambda: q_proj_half(1, 0, 0, hq)),
        (10, lambda: q_proj_half(1, 0, 1, hq)),
        (11, lambda: q_rope(1, 0, hq, False)),
        (12, lambda: norm_mul(5)),
        (13, lambda: k_proj(4, 0, hs0[4])),
        (14, lambda: k_rope_a(4, 0, hs0[4], False)),
        (15, lambda: k_rope_b(4, 0, hs0[4], False)),
        (16, lambda: k_proj(5, 0, hs0[5])),
        (17, lambda: k_rope_a(5, 0, hs0[5], False)),
        (18, lambda: k_rope_b(5, 0, hs0[5], False)),
        (19, lambda: norm_mul(7)),
        (20, lambda: k_proj(6, 0, hs0[6])),
        (21, lambda: k_rope_a(6, 0, hs0[6], False)),
        (22, lambda: k_rope_b(6, 0, hs0[6], False)),
        (24, lambda: k_proj(7, 0, hs0[7])),
        (25, lambda: k_rope_a(7, 0, hs0[7], False)),
        (26, lambda: k_rope_b(7, 0, hs0[7], False)),
    ]
    attention_block(0, 0, cl, pinned_v=0)

    # ================= attention block 2: (hp0, qc1) =================
    # k1 chunks 0-3 (rest in block 3), Q(hp1,qc0), V(hp1) priming.
    hs = [[None] for _ in range(8)]
    hq = [None]
    cl = [
        (0, lambda: k_proj(0, 1, hs[0])),
        (1, lambda: k_rope_a(0, 1, hs[0], False)),
        (2, lambda: k_rope_b(0, 1, hs[0], False)),
        (4, lambda: k_proj(1, 1, hs[1])),
        (5, lambda: k_rope_a(1, 1, hs[1], False)),
        (6, lambda: k_rope_b(1, 1, hs[1], False)),
        (8, lambda: q_proj_half(0, 1, 0, hq)),
        (9, lambda: q_proj_half(0, 1, 1, hq)),
        (10, lambda: q_rope(0, 1, hq, False)),
        (12, lambda: k_proj(2, 1, hs[2])),
        (13, lambda: k_rope_a(2, 1, hs[2], False)),
        (14, lambda: k_rope_b(2, 1, hs[2], False)),
        (16, lambda: k_proj(3, 1, hs[3])),
        (17, lambda: k_rope_a(3, 1, hs[3], False)),
        (18, lambda: k_rope_b(3, 1, hs[3], False)),
        (27, lambda: v_chunk(0, 1, False)),
        (29, lambda: v_chunk(1, 1, False)),
    ]
    attention_block(0, 1, cl)

    # ================= attention block 3: (hp1, qc0) =================
    # k1 chunks 4-7 JIT (chunk sf feeds scores iters 4sf+), Q(hp1,qc1).
    hq = [None]
    cl = [
        (0, lambda: k_proj(4, 1, hs[4])),
        (1, lambda: k_rope_a(4, 1, hs[4], False)),
        (2, lambda: k_rope_b(4, 1, hs[4], False)),
        (4, lambda: q_proj_half(1, 1, 0, hq)),
        (5, lambda: q_proj_half(1, 1, 1, hq)),
        (6, lambda: q_rope(1, 1, hq, False)),
        (8, lambda: k_proj(5, 1, hs[5])),
        (9, lambda: k_rope_a(5, 1, hs[5], False)),
        (10, lambda: k_rope_b(5, 1, hs[5], False)),
        (13, lambda: k_proj(6, 1, hs[6])),
        (14, lambda: k_rope_a(6, 1, hs[6], False)),
        (15, lambda: k_rope_b(6, 1, hs[6], False)),
        (18, lambda: k_proj(7, 1, hs[7])),
        (19, lambda: k_rope_a(7, 1, hs[7], False)),
        (20, lambda: k_rope_b(7, 1, hs[7], False)),
    ]
    attention_block(1, 0, cl, pinned_v=1)

    # ================= attention block 4: (hp1, qc1) + outproj(qc0) ====
    cl = []
    osbs = {}
    for q8 in range(4):
        osbs[q8] = None
        for oc in range(2):
            ph = [None]
            t = 4 * q8 + 2 * oc

            def mk_a(q8=q8, oc=oc, ph=ph):
                return lambda: outproj_a(q8, oc, ph)

            def mk_b(q8=q8, oc=oc, ph=ph):
                def f():
                    if oc == 0:
                        osbs[q8] = work.tile([P, 1024], F32, tag="osb", name="osb")
                    outproj_b(q8, oc, ph, osbs[q8], vec_eng=True)
                return f

            cl.append((t, mk_a()))
            cl.append((t + 1, mk_b()))
    attention_block(1, 1, cl)

    # ================= tail: outproj(qc1) =================
    for q8 in range(4, 8):
        osb = work.tile([P, 1024], F32, tag="osb")
        for oc in range(2):
            ph = [None]
            outproj_a(q8, oc, ph)
            outproj_b(q8, oc, ph, osb, vec_eng=(oc == 0))


def build_program():
    nc = bacc.Bacc("TRN2", target_bir_lowering=False, debug=False)
    io = {}

    def inp(name, shape, dtype=F32):
        io[name] = nc.dram_tensor(name, list(shape), dtype, kind="ExternalInput").ap()

    inp("x_b", [P, 4, S], BF16)
    inp("x_byp", [P, 4, LQ], MM_F32)
    inp("wq", [P, 16, NH], BF16)
    inp("wk", [P, 4, NH], BF16)
    inp("wv", [P, 4, NH], BF16)
    inp("biases", [P, 6])
    inp("wo", [P, 2, DLAT], BF16)
    inp("wb", [P, 4, DLAT], MM_F32)
    inp("csq", [P, 2, LQ], BF16)
    inp("csk", [P, 2, S], BF16)
    inp("rotm", [P, P], BF16)
    io["out_partial"] = nc.dram_tensor(
        "out_partial", [LQ, DLAT], F32, kind="ExternalOutput"
    ).ap()

    with tile.TileContext(nc) as tc:
        with ExitStack() as ctx:
            _kernel_body(ctx, tc, io)
    nc.compile()
    return nc


# s' = (s % 4) * LQ + s // 4  (sub-major permutation of the byte sequence)
_PERM = (np.arange(S) % BPL) * LQ + np.arange(S) // BPL
_IPERM = np.argsort(_PERM)  # original s for each s'


def _chunked_rows(w, dtype):
    """[C*128, N] -> [128, C, N] (partition-major chunks for direct DMA)."""
    c = w.shape[0] // P
    return np.ascontiguousarray(w.reshape(c, P, -1).transpose(1, 0, 2).astype(dtype))


def _rope_tables(pos):
    half = DQK // 2
    invfreq = ROPE_BASE ** (-np.arange(half, dtype=np.float64) / half)
    ang = pos[:, None].astype(np.float64) * invfreq[None, :]
    cos = np.cos(ang)
    sin = np.sin(ang)
    cos64 = np.concatenate([cos, cos], axis=1).T  # [64, L]
    sin64 = np.concatenate([-sin, sin], axis=1).T
    cosT = np.concatenate([cos64, cos64], axis=0)
    sinT = np.concatenate([sin64, sin64], axis=0)
    return cosT, sinT


def _tf32(a):
    u = np.ascontiguousarray(np.asarray(a, dtype=np.float32)).view(np.uint32)
    lsb = (u >> np.uint32(13)) & np.uint32(1)
    u = (u + np.uint32(0x0FFF) + lsb) & np.uint32(0xFFFFE000)
    return u.view(np.float32)


def _bf16(a):
    import ml_dtypes

    return np.ascontiguousarray(np.asarray(a).astype(ml_dtypes.bfloat16))


def make_in_map(core, inputs):
    b, hg = core // 4, core % 4
    x = np.asarray(inputs["x"], dtype=np.float32)
    nw = np.asarray(inputs["norm_w"], dtype=np.float32)
    wq_w = np.asarray(inputs["wq_w"], dtype=np.float32)
    wq_b = np.asarray(inputs["wq_b"], dtype=np.float32)
    wkv_w = np.asarray(inputs["wkv_w"], dtype=np.float32)
    wkv_b = np.asarray(inputs["wkv_b"], dtype=np.float32)
    wout_w = np.asarray(inputs["wout_w"], dtype=np.float32)
    wbyp_w = np.asarray(inputs["wbyp_w"], dtype=np.float32)

    import ml_dtypes

    BF = ml_dtypes.bfloat16
    nsl = slice(hg * NH, (hg + 1) * NH)
    vsl = slice(H * DQK + hg * NH, H * DQK + (hg + 1) * NH)
    wq_c = wq_w * np.tile(nw, BPL)[:, None]
    wkv_c = wkv_w * nw[:, None]

    cosq, sinq = _rope_tables(np.arange(LQ) * float(BPL))
    cosk, sink = _rope_tables(_IPERM.astype(np.float64))  # k tables in s' order

    rotm = np.zeros((P, P), dtype=np.float32)
    for m in range(P):
        blk, d = (m // 64) * 64, m % 64
        rotm[blk + (d + 32) % 64, m] = 1.0

    xp = x[b][_IPERM, :]  # rows reordered to s' order
    return {
        "x_b": _bf16(xp.T.reshape(4, P, S).transpose(1, 0, 2)),
        "x_byp": _tf32(
            np.ascontiguousarray(x[b, hg::BPL, :].T.reshape(4, P, LQ).transpose(1, 0, 2))
        ),
        "wq": _chunked_rows(wq_c[:, nsl], BF),
        "wk": _chunked_rows(wkv_c[:, nsl], BF),
        "wv": _chunked_rows(wkv_c[:, vsl], BF),
        "biases": np.ascontiguousarray(
            np.concatenate(
                [
                    wkv_b[nsl].reshape(2, P).T,
                    wq_b[nsl].reshape(2, P).T,
                    wkv_b[vsl].reshape(2, P).T,
                ],
                axis=1,
            )
        ),
        "wo": _chunked_rows(wout_w[nsl, :], BF),
        "wb": _tf32(_chunked_rows(wbyp_w[hg * D : (hg + 1) * D, :], np.float32)),
        "csq": _bf16(np.stack([cosq, sinq], axis=1)),
        "csk": _bf16(np.stack([cosk, sink], axis=1)),
        "rotm": _bf16(rotm),
    }


_nc_cache = None


def _get_program():
    global _nc_cache
    if _nc_cache is None:
        _nc_cache = build_program()
    return _nc_cache


def run_device(inputs, trace=False):
    nc = _get_program()
    in_maps = [make_in_map(c, inputs) for c in range(NCORES)]
    res = run_bass_kernel_spmd(nc, in_maps, core_ids=list(range(NCORES)), trace=trace)
    return res


def assemble(parts, inputs):
    wout_b = np.asarray(inputs["wout_b"], dtype=np.float32)
    out = np.zeros((B, LQ, DLAT), dtype=np.float64)
    for c in range(NCORES):
        out[c // 4] += np.asarray(parts[c], dtype=np.float64)
    out += wout_b[None, None, :].astype(np.float64)
    return out.astype(np.float32)


def kernel(**inputs):
    res = run_device(inputs)
    parts = [r["out_partial"] for r in res.results]
    return assemble(parts, inputs)
